# revision 11
# baseline (speedup 1.0000x reference)
"""Trainium2 Bass kernel for the BERT span-pair classifier problem.

Computes, for B=2 batches over a 252x252 span-pair grid:
    h    = relu(Ai[i] + Aj[j] + ind(i,j)*w1c + b1)        # [770] per pair
    out  = h @ W2.T + b2                                   # [36]  per pair
    out  = where(span_mask >= 1, out, 0)
    res  = log_softmax(out over the 63504 pairs)           # per (batch, label)
    return res transposed to [B, 36, L*L]

Strategy (8 NeuronCores, SPMD single program):
  - The 504 (batch, row-i) rows are distributed over 8 cores, grouped into 4
    slot segments (in-span b0, in-span b1, off-span b0, off-span b1) so that
    every core runs an identical instruction template per slot.  In-span
    segments are assigned round-robin (row = s_b + 8*k + core) so the only
    runtime-varying quantity (the span boundary j = i) is `static + core_id`.
  - h is produced in [hid-chunk(128) x j] layout by fused tensor_scalar
    (add per-row bias, relu) ops; the span indicator is realized by a second
    windowed overwrite sourced from BjE = Bj0 + w1c * [j <= e].
  - The 770->36 matmul streams h as the moving operand against stationary
    W2^T chunks; mask, b2 and a -BIG invalid-pair offset are folded in as
    extra contraction rows.
  - exp + per-tile softmax sums come from ACT activation(Exp, accum_out=..);
    the global log-sum-exp is combined with a tiny [36,4] AllReduce.
"""

import math
import os
from contextlib import ExitStack

import numpy as np

import concourse.bass as bass
import concourse.bacc as bacc
import concourse.tile as tile
from concourse import mybir
from concourse._compat import with_exitstack
from concourse.bass_utils import run_bass_kernel_spmd

L = 252
HID = 768
MLP = 770
NLAB = 36
B = 2
NC = 8
KC = 6           # full 128-row hid chunks (6*128 = 768)
BIGNEG = -30.0   # makes exp(invalid pair) ~ 0 while staying fp32-exact

FP32 = mybir.dt.float32
BF16 = mybir.dt.bfloat16
AF = mybir.ActivationFunctionType
ALU = mybir.AluOpType


def _ceil2(x):
    return x + (x & 1)


def plan_slots(spans):
    """Compute the slot layout shared by host + device.

    Returns dict with per-segment (kind, batch, start_slot, nslots, rows).
    In-span segments: core c, slot p holds global row  s_b + 8*(p-start) + c.
    Off-span segments: arbitrary round-robin over the listed rows.
    """
    segs = []
    slot = 0
    for b in range(B):
        s, e = spans[b]
        n = e - s + 1
        nsl = _ceil2(math.ceil(n / NC))
        segs.append(dict(kind="in", b=b, start=slot, nslots=nsl, s=s, e=e, count=n))
        slot += nsl
    for b in range(B):
        s, e = spans[b]
        rows = [r for r in range(L) if r < s or r > e]
        nsl = _ceil2(math.ceil(len(rows) / NC))
        segs.append(dict(kind="off", b=b, start=slot, nslots=nsl, rows=rows,
                         count=len(rows)))
        slot += nsl
    nslot = slot
    assert nslot % 2 == 0
    return segs, nslot


def slot_map_for_core(segs, nslot, c):
    """-> list over slots of (batch, global_row) or None for padding."""
    m = [None] * nslot
    for sg in segs:
        for k in range(sg["nslots"]):
            idx = NC * k + c
            p = sg["start"] + k
            if idx < sg["count"]:
                if sg["kind"] == "in":
                    m[p] = (sg["b"], sg["s"] + idx)
                else:
                    m[p] = (sg["b"], sg["rows"][idx])
    return m


def build_kernel(spans, segs, nslot, engine_plan):
    """Returns a tile-kernel closure over the compile-time span values."""
    ntile = nslot // 2
    # batch of each slot (static, same on every core)
    slot_batch = []
    slot_seg = []
    for sg in segs:
        for k in range(sg["nslots"]):
            slot_batch.append(sg["b"])
            slot_seg.append(sg)
    # tiles owned by each batch (for the LSE reduction) -- contiguous runs
    btiles = {b: [t for t in range(ntile)
                  if slot_batch[2 * t] == b] for b in range(B)}
    for b in range(B):
        ts_ = btiles[b]
        assert all(slot_batch[2 * t] == slot_batch[2 * t + 1] for t in ts_)

    HW = 1024          # per-slot h scratch width (252 real + padding)
    SLOTW = 512        # per-slot region inside an h tile

    @with_exitstack
    def kern(ctx: ExitStack, tc: tile.TileContext, outs, ins):
        nc = tc.nc
        w1iT = ins["w1iT"]      # [768, 770] bf16 (DRAM)
        w1jT = ins["w1jT"]      # [768, 770] bf16
        w1c = ins["w1c"]        # [770] f32
        b1 = ins["b1"]          # [770] f32
        w2T6 = ins["w2T6"]      # [768, 36] bf16  (chunks 0..5 of W2^T)
        w2Tt = ins["w2Tt"]      # [4, 36] bf16    (rows 768,769, b2, ones*BIGNEG)
        vecsT = ins["vecsT"]    # [768, 504] bf16  (both batches, all j)
        myvT = ins["myvT"]      # [768, nslot] bf16 (per-core slot rows)
        maskb = ins["maskb"]    # [2, nslot*512] bf16: row0 = m, row1 = 1-m
        maskf = ins["maskf"]    # [nslot, 252] f32 (0/1 per-slot mask rows)
        flags = ins["flags"]    # [nslot, 2] f32: col b = 1 if slot is real row of b
        e2f = ins["e2f"]        # [128, 2] f32: col b = 1 if this core owns row s_b
        outd = outs["out"]      # [36, nslot*252] f32
        lseo = outs["lse"]      # [36, 4] f32 (debug: partial/total sums)

        fp = ctx.enter_context(tc.tile_pool(name="fp", bufs=1))
        prep_ps = ctx.enter_context(tc.tile_pool(name="prep_ps", bufs=1, space="PSUM"))
        main_ps = ctx.enter_context(tc.tile_pool(name="main_ps", bufs=4, space="PSUM"))
        cnt_ps = ctx.enter_context(tc.tile_pool(name="cnt_ps", bufs=1, space="PSUM"))
        hp = [ctx.enter_context(tc.tile_pool(name=f"h{c}", bufs=3)) for c in range(7)]
        mrp = ctx.enter_context(tc.tile_pool(name="mrp", bufs=3))
        esp = ctx.enter_context(tc.tile_pool(name="esp", bufs=2))
        dram = ctx.enter_context(tc.tile_pool(name="dram", bufs=1, space="DRAM"))

        # ---- persistent SBUF ----
        s_w1i = [fp.tile([128, MLP], BF16, tag=f"w1i{k}", name=f"w1i{k}") for k in range(KC)]
        s_w1j = [fp.tile([128, MLP], BF16, tag=f"w1j{k}", name=f"w1j{k}") for k in range(KC)]
        s_vT = [fp.tile([128, 2 * L], BF16, tag=f"vT{k}", name=f"vT{k}") for k in range(KC)]
        s_myv = [fp.tile([128, nslot], BF16, tag=f"myv{k}", name=f"myv{k}") for k in range(KC)]
        s_w2 = fp.tile([128, 6 * NLAB], BF16)      # W2T chunks 0..5 side by side
        s_w2h = fp.tile([2, NLAB], BF16)           # W2T rows 768,769
        s_w2m = fp.tile([2, NLAB], BF16)           # [b2 ; BIGNEG] rows
        s_w1c = fp.tile([128, 7], FP32)            # w1c per-chunk columns
        s_b1 = fp.tile([128, 7], FP32)
        s_e2f = fp.tile([128, 2], FP32)
        CS = [128] * KC + [2]                      # chunk sizes of 770
        # Bj0 (bf16, A-source), BjE (f32, B-source), per batch padded to 512
        s_bj0 = [fp.tile([128, B * 512], BF16, tag=f"bj0_{c}", name=f"bj0_{c}") for c in range(7)]
        s_bje = [fp.tile([128, B * 512], FP32, tag=f"bje_{c}", name=f"bje_{c}") for c in range(7)]
        s_ai = [fp.tile([128, nslot], FP32, tag=f"ai{c}", name=f"ai{c}") for c in range(7)]
        s_aiE2 = [fp.tile([128, B], FP32, tag=f"aiE2{c}", name=f"aiE2{c}") for c in range(7)]
        s_out = fp.tile([NLAB, nslot * L], FP32)
        s_acc = fp.tile([NLAB, ntile], FP32)
        s_mrow = fp.tile([nslot, L], FP32)
        s_flag = fp.tile([nslot, 2], FP32)
        s_ones = fp.tile([nslot, NLAB], BF16)
        s_sums = fp.tile([NLAB, 4], FP32)          # [cnt_b0,cnt_b1,sum_b0,sum_b1]
        s_nlse = fp.tile([NLAB, B], FP32)

        # ---- load constants ----
        for k in range(KC):
            nc.sync.dma_start(out=s_w1i[k], in_=w1iT[128 * k:128 * (k + 1), :])
            nc.sync.dma_start(out=s_w1j[k], in_=w1jT[128 * k:128 * (k + 1), :])
            nc.sync.dma_start(out=s_vT[k], in_=vecsT[128 * k:128 * (k + 1), :])
            nc.sync.dma_start(out=s_myv[k], in_=myvT[128 * k:128 * (k + 1), :])
            nc.sync.dma_start(out=s_w2[:, NLAB * k:NLAB * (k + 1)],
                              in_=w2T6[128 * k:128 * (k + 1), :])
        nc.sync.dma_start(out=s_w2h, in_=w2Tt[0:2, :])
        nc.sync.dma_start(out=s_w2m, in_=w2Tt[2:4, :])
        # w1c/b1 [770] -> [128, 7] (partition p, col c) = v[128*c + p]; the
        # last column only has 2 valid rows -- load cols 0..5 strided + tail.
        w1c2 = w1c[0:HID].rearrange("(c p) -> p c", p=128)
        nc.sync.dma_start(out=s_w1c[:, 0:KC], in_=w1c2)
        nc.sync.dma_start(out=s_w1c[0:2, KC:7], in_=w1c[HID:MLP].rearrange("(p o) -> p o", o=1))
        b12 = b1[0:HID].rearrange("(c p) -> p c", p=128)
        nc.sync.dma_start(out=s_b1[:, 0:KC], in_=b12)
        nc.sync.dma_start(out=s_b1[0:2, KC:7], in_=b1[HID:MLP].rearrange("(p o) -> p o", o=1))
        nc.sync.dma_start(out=s_e2f, in_=e2f)
        nc.sync.dma_start(out=s_mrow, in_=maskf)
        nc.sync.dma_start(out=s_flag, in_=flags)
        nc.vector.memset(s_ones, 1.0)

        # ---- prep: AjT -> Bj0/BjE, AiT ----
        for c in range(7):
            cs = CS[c]
            mlo = 128 * c
            # Bj (= Aj + b1) for all 504 (b, j) columns
            psA = prep_ps.tile([128, 2 * L], FP32, tag="psA", name=f"psA{c}")
            for k in range(KC):
                nc.tensor.matmul(psA[:cs, :], s_w1j[k][:, mlo:mlo + cs], s_vT[k],
                                 start=(k == 0), stop=(k == KC - 1))
            bje = s_bje[c]
            for b in range(B):
                nc.vector.tensor_scalar(
                    out=bje[:cs, 512 * b:512 * b + L],
                    in0=psA[:cs, L * b:L * (b + 1)],
                    scalar1=s_b1[:cs, c:c + 1], scalar2=None, op0=ALU.add)
            # bf16 copy (A-source) before the w1c prefix gets added
            for b in range(B):
                nc.gpsimd.tensor_copy(out=s_bj0[c][:cs, 512 * b:512 * b + L],
                                      in_=bje[:cs, 512 * b:512 * b + L])
                # pad region: keep finite values (copy of col 0)
                nc.gpsimd.memset(s_bj0[c][:cs, 512 * b + L:512 * (b + 1)], 0.0)
                nc.vector.memset(bje[:cs, 512 * b + L:512 * (b + 1)], 0.0)
            # BjE = Bj0 + w1c on j <= e_b
            for b in range(B):
                e = spans[b][1]
                nc.vector.tensor_scalar(
                    out=bje[:cs, 512 * b:512 * b + e + 1],
                    in0=bje[:cs, 512 * b:512 * b + e + 1],
                    scalar1=s_w1c[:cs, c:c + 1], scalar2=None, op0=ALU.add)
            # AiT for my slots
            psI = prep_ps.tile([128, nslot], FP32, tag="psI", name=f"psI{c}")
            for k in range(KC):
                nc.tensor.matmul(psI[:cs, :], s_w1i[k][:, mlo:mlo + cs], s_myv[k],
                                 start=(k == 0), stop=(k == KC - 1))
            nc.vector.tensor_copy(out=s_ai[c][:cs, :], in_=psI[:cs, :])
            # E2 bias: Ai[slot_e2] + w1c * own_flag  (per batch)
            for b in range(B):
                p_e2 = next(sg for sg in segs
                            if sg["kind"] == "in" and sg["b"] == b)["start"]
                nc.vector.tensor_tensor(
                    out=s_aiE2[c][:cs, b:b + 1],
                    in0=s_w1c[:cs, c:c + 1], in1=s_e2f[:cs, b:b + 1],
                    op=ALU.mult)
                nc.vector.tensor_tensor(
                    out=s_aiE2[c][:cs, b:b + 1],
                    in0=s_aiE2[c][:cs, b:b + 1], in1=s_ai[c][:cs, p_e2:p_e2 + 1],
                    op=ALU.add)

        pid = {}

        def eng_pid(eng):
            if eng not in pid:
                pid[eng] = eng.partition_id()
            return pid[eng]

        def ts_relu(eng, out, in0, sc):
            if eng is nc.scalar:
                nc.scalar.activation(out, in0, AF.Relu, bias=sc, scale=1.0)
            else:
                eng.tensor_scalar(out=out, in0=in0, scalar1=sc, scalar2=0.0,
                                  op0=ALU.add, op1=ALU.max)

        # ---- main loop over 2-slot tiles ----
        for t in range(ntile):
            hts = []
            for c in range(7):
                cs = CS[c]
                ht = hp[c].tile([cs, HW], BF16, tag=f"ht{c}", name=f"ht{c}_{t}")
                hts.append(ht)
                for sl in range(2):
                    p = 2 * t + sl
                    b = slot_batch[p]
                    sg = slot_seg[p]
                    base = SLOTW * sl
                    engA, engB = engine_plan(t, c, p)
                    # segment A: whole row from Bj0 (correct outside [i, e])
                    ts_relu(engA, ht[:cs, base:base + 256],
                            s_bj0[c][:cs, 512 * b:512 * b + 256],
                            s_ai[c][:cs, p:p + 1])
                    if sg["kind"] == "in":
                        # segment B: overwrite [i, i+256) from BjE (adds w1c
                        # on [i, e]; equals Bj0 past e)
                        kk = p - sg["start"]
                        ioff = sg["s"] + NC * kk + eng_pid(engB)
                        ts_relu(engB, ht[:cs, bass.ds(base + ioff, 256)],
                                s_bje[c][:cs, bass.ds(512 * b + ioff, 256)],
                                s_ai[c][:cs, p:p + 1])
                    # E2 pixel: slot holding row s_b gets +2*w1c at j=e_b
                    if sg["kind"] == "in" and p == sg["start"]:
                        e = sg["e"]
                        ts_relu(nc.vector, ht[:cs, base + e:base + e + 1],
                                s_bje[c][:cs, 512 * b + e:512 * b + e + 1],
                                s_aiE2[c][:cs, b:b + 1])

            # matmul: psum[36, 504] over 6 full chunks + tail + mask rows
            ps = main_ps.tile([NLAB, 2 * L], FP32, tag="ps", name=f"ps{t}")
            rhs6 = [hts[c][:, :].rearrange("p (s w) -> p s w", w=SLOTW)
                    [:, :, 0:L] for c in range(6)]
            for c in range(6):
                nc.tensor.matmul(ps, s_w2[:, NLAB * c:NLAB * (c + 1)], rhs6[c],
                                 start=(c == 0), stop=False)
            rhs_t = hts[6][:, :].rearrange("p (s w) -> p s w", w=SLOTW)[:, :, 0:L]
            nc.tensor.matmul(ps, s_w2h, rhs_t, start=False, stop=False)
            mb = maskb[:, HW * t:HW * (t + 1)].rearrange("p (s w) -> p s w", w=SLOTW)
            mrhs = mrp.tile([2, HW], BF16, tag="mb", name=f"mb{t}")
            nc.sync.dma_start(out=mrhs, in_=maskb[:, HW * t:HW * (t + 1)])
            nc.tensor.matmul(
                ps, s_w2m,
                mrhs[:, :].rearrange("p (s w) -> p s w", w=SLOTW)[:, :, 0:L],
                start=False, stop=True)

            # exp + per-tile softmax sum (invalid pairs contribute ~0)
            esc = esp.tile([NLAB, 2 * L], FP32, tag="esc", name=f"esc{t}")
            nc.scalar.activation(esc, ps, AF.Exp, accum_out=s_acc[:, t:t + 1])

            # masked values -> out buffer (mask replicated over 36 partitions)
            mrep = mrp.tile([NLAB, 2 * L], FP32, tag="mrep", name=f"mrep{t}")
            msrc = maskf[2 * t:2 * t + 2, :]
            nc.sync.dma_start(
                out=mrep,
                in_=bass.AP(tensor=msrc.tensor, offset=msrc.offset,
                            ap=[[0, NLAB], [L, 2], [1, L]]))
            nc.vector.tensor_tensor(out=s_out[:, 2 * L * t:2 * L * (t + 1)],
                                    in0=ps, in1=mrep, op=ALU.mult)

        # ---- LSE: counts + tile sums + AllReduce + log ----
        srow = fp.tile([nslot, 1], FP32)
        nc.vector.tensor_reduce(out=srow, in_=s_mrow, axis=mybir.AxisListType.X, op=ALU.add)
        vb = fp.tile([nslot, B], BF16)
        vf = fp.tile([nslot, B], FP32)
        for b in range(B):
            nc.vector.tensor_scalar(out=vf[:, b:b + 1], in0=srow,
                                    scalar1=-1.0, scalar2=float(L),
                                    op0=ALU.mult, op1=ALU.add)
        nc.vector.tensor_tensor(out=vb, in0=vf, in1=s_flag, op=ALU.mult)
        cps = cnt_ps.tile([NLAB, B], FP32, tag="cnt", name="cnt")
        nc.tensor.matmul(cps, s_ones[:, 0:NLAB], vb, start=True, stop=True)
        nc.vector.tensor_copy(out=s_sums[:, 0:B], in_=cps)
        for b in range(B):
            rs = btiles[b]
            # contiguous runs of tiles for this batch
            runs = []
            st = rs[0]
            for a, bb in zip(rs, rs[1:] + [None]):
                if bb != a + 1:
                    runs.append((st, a))
                    st = bb
            t0, t1 = runs[0]
            nc.vector.tensor_reduce(out=s_sums[:, 2 + b:3 + b],
                                    in_=s_acc[:, t0:t1 + 1],
                                    axis=mybir.AxisListType.X, op=ALU.add)
            for (u0, u1) in runs[1:]:
                tmp = fp.tile([NLAB, 1], FP32, tag=f"tr{b}", name=f"tr{b}_{u0}")
                nc.vector.tensor_reduce(out=tmp, in_=s_acc[:, u0:u1 + 1],
                                        axis=mybir.AxisListType.X, op=ALU.add)
                nc.vector.tensor_tensor(out=s_sums[:, 2 + b:3 + b],
                                        in0=s_sums[:, 2 + b:3 + b], in1=tmp,
                                        op=ALU.add)
        # partial = tilesum + invalid count
        for b in range(B):
            nc.vector.tensor_tensor(out=s_sums[:, 2 + b:3 + b],
                                    in0=s_sums[:, 2 + b:3 + b],
                                    in1=s_sums[:, b:b + 1], op=ALU.add)
            nc.vector.memset(s_sums[:, b:b + 1], 0.0)
        cc_in = dram.tile([NLAB, 4], FP32, name="cc_in")
        cc_out = nc.dram_tensor("cc_out", [NLAB, 4], FP32, kind="Internal",
                                addr_space="Shared").ap()
        nc.sync.dma_start(out=cc_in, in_=s_sums)
        nc.gpsimd.collective_compute(
            "AllReduce", ALU.add, replica_groups=[list(range(NC))],
            ins=[cc_in[:]], outs=[cc_out], cc_dim="Partition")
        allsum = fp.tile([NLAB, 4], FP32)
        nc.sync.dma_start(out=allsum, in_=cc_out)
        nc.sync.dma_start(out=lseo, in_=allsum)
        nc.scalar.activation(s_nlse, allsum[:, 2:4], AF.Ln)
        nc.vector.tensor_scalar(out=s_nlse, in0=s_nlse, scalar1=-1.0,
                                scalar2=None, op0=ALU.mult)

        # ---- final: out - LSE, store ----
        for t in range(ntile):
            b = slot_batch[2 * t]
            seg = s_out[:, 2 * L * t:2 * L * (t + 1)]
            nc.vector.tensor_scalar(out=seg, in0=seg,
                                    scalar1=s_nlse[:, b:b + 1], scalar2=None,
                                    op0=ALU.add)
            nc.sync.dma_start(out=outd[:, 2 * L * t:2 * L * (t + 1)], in_=seg)

    return kern


def default_engine_plan(nc_getter):
    """Static engine assignment for the h-generation instructions."""
    def plan(t, c, p):
        nc = nc_getter()
        # A-instructions: spread chunks across DVE / GPSIMD; B: ACT + DVE
        engA = nc.vector if c % 2 == 0 else nc.gpsimd
        engB = nc.scalar if c % 3 != 0 else nc.vector
        return engA, engB
    return plan


def kernel(**inputs) -> np.ndarray:
    hidden = np.asarray(inputs["hidden"], dtype=np.float32)
    pred_spans = np.asarray(inputs["pred_spans"]).astype(np.int64)
    span_mask = np.asarray(inputs["span_mask"]).astype(np.int32)
    W1 = np.asarray(inputs["W1"], dtype=np.float32)
    b1 = np.asarray(inputs["b1"], dtype=np.float32)
    W2 = np.asarray(inputs["W2"], dtype=np.float32)
    b2 = np.asarray(inputs["b2"], dtype=np.float32)

    spans = [(int(pred_spans[b, 0]), int(pred_spans[b, 1])) for b in range(B)]
    segs, nslot = plan_slots(spans)
    ntile = nslot // 2

    vecs = hidden[:, 1:L + 1, :]                       # [B, L, 768]
    vecsT = np.concatenate([vecs[0].T, vecs[1].T], axis=1)   # [768, 504]
    W1T = W1.T                                          # [1537, 770]
    w1iT = np.ascontiguousarray(W1T[0:HID]).astype(np.float32)
    w1jT = np.ascontiguousarray(W1T[HID:2 * HID]).astype(np.float32)
    w1c = np.ascontiguousarray(W1T[2 * HID]).astype(np.float32)
    W2T = np.ascontiguousarray(W2.T)                    # [770, 36]
    w2T6 = W2T[0:HID]
    w2Tt = np.stack([W2T[768], W2T[769], b2,
                     np.full(NLAB, BIGNEG, np.float32)], axis=0)

    maskf_full = span_mask.astype(np.float32).clip(0, 1)

    in_maps = []
    slot_maps = []
    for c in range(NC):
        sm = slot_map_for_core(segs, nslot, c)
        slot_maps.append(sm)
        myv = np.zeros((HID, nslot), np.float32)
        maskf = np.zeros((nslot, L), np.float32)
        flags = np.zeros((nslot, 2), np.float32)
        for p, ent in enumerate(sm):
            if ent is None:
                continue
            b, r = ent
            myv[:, p] = vecs[b, r]
            maskf[p] = maskf_full[r]
            flags[p, b] = 1.0
        maskb = np.zeros((2, nslot * 512), np.float32)
        for t in range(ntile):
            for sl in range(2):
                p = 2 * t + sl
                maskb[0, 1024 * t + 512 * sl:1024 * t + 512 * sl + L] = maskf[p]
                maskb[1, 1024 * t + 512 * sl:1024 * t + 512 * sl + L] = \
                    1.0 - maskf[p]
        e2f = np.zeros((128, 2), np.float32)
        for b in range(B):
            if c == 0:
                e2f[:, b] = 1.0      # row s_b lives on core 0 (slot seg start)
        in_maps.append({
            "w1iT": w1iT, "w1jT": w1jT, "w1c": w1c, "b1": b1,
            "w2T6": w2T6, "w2Tt": w2Tt,
            "vecsT": vecsT, "myvT": myv,
            "maskb": maskb, "maskf": maskf, "flags": flags, "e2f": e2f,
        })

    # ---- build program ----
    nc = bacc.Bacc("TRN2", target_bir_lowering=False, debug=False,
                   enable_asserts=False, num_devices=NC)

    def mk(name, arr, dt):
        return nc.dram_tensor(name, list(arr.shape), dt, kind="ExternalInput").ap()

    ml_bf = lambda n, a: mk(n, a, BF16)
    ml_f32 = lambda n, a: mk(n, a, FP32)
    ex = in_maps[0]
    ins_aps = {
        "w1iT": ml_bf("w1iT", ex["w1iT"]), "w1jT": ml_bf("w1jT", ex["w1jT"]),
        "w1c": ml_f32("w1c", ex["w1c"]), "b1": ml_f32("b1", ex["b1"]),
        "w2T6": ml_bf("w2T6", ex["w2T6"]), "w2Tt": ml_bf("w2Tt", ex["w2Tt"]),
        "vecsT": ml_bf("vecsT", ex["vecsT"]), "myvT": ml_bf("myvT", ex["myvT"]),
        "maskb": ml_bf("maskb", ex["maskb"]), "maskf": ml_f32("maskf", ex["maskf"]),
        "flags": ml_f32("flags", ex["flags"]), "e2f": ml_f32("e2f", ex["e2f"]),
    }
    outs_aps = {
        "out": nc.dram_tensor("out", [NLAB, nslot * L], FP32,
                              kind="ExternalOutput").ap(),
        "lse": nc.dram_tensor("lse", [NLAB, 4], FP32,
                              kind="ExternalOutput").ap(),
    }

    plan = default_engine_plan(lambda: nc)
    kern = build_kernel(spans, segs, nslot, plan)
    with tile.TileContext(nc) as t:
        kern(t, outs_aps, ins_aps)
    nc.compile()

    # bf16-cast the bf16 inputs host-side
    def cast_maps(m):
        out = {}
        for k, v in m.items():
            dt = ins_aps[k].dtype
            if dt == BF16:
                out[k] = v.astype(mybir.dt.np(BF16))
            else:
                out[k] = v.astype(np.float32)
        return out

    in_maps_c = [cast_maps(m) for m in in_maps]

    if os.environ.get("BK_BUILD_ONLY"):
        print("BUILD OK")
        return np.zeros((B, NLAB, L * L), np.float32)

    if os.environ.get("BK_SIM"):
        from concourse.bass_interp import MultiCoreSim

        sim = MultiCoreSim(nc, num_cores=NC, require_finite=False,
                           require_nnan=False)
        for c, cs in sim.cores.items():
            for name, arr in in_maps_c[c].items():
                cs.tensor(name)[:] = arr
            if nc.partition_id_tensor is not None:
                cs.tensor(nc.partition_id_tensor.name)[:] = np.array(
                    [[c]], dtype=np.uint32)
        sim.simulate(check_with_hw=False)

        class _R:
            results = [{"out": np.asarray(sim.cores[c].tensor("out")),
                        "lse": np.asarray(sim.cores[c].tensor("lse"))}
                       for c in range(NC)]
        res = _R()
    else:
        trace = bool(int(os.environ.get("BK_TRACE", "0")))
        res = run_bass_kernel_spmd(nc, in_maps_c, core_ids=list(range(NC)),
                                   trace=trace)
        if trace and res.exec_time_ns is not None:
            print(f"HW exec time: {res.exec_time_ns} ns")

    # ---- unshard ----
    out_full = np.zeros((B, NLAB, L * L), np.float32)
    for c in range(NC):
        oc = res.results[c]["out"]          # [36, nslot*252]
        for p, ent in enumerate(slot_maps[c]):
            if ent is None:
                continue
            b, r = ent
            out_full[b, :, L * r:L * (r + 1)] = oc[:, L * p:L * (p + 1)]
    return out_full


# revision 13
# speedup vs baseline: 3.3303x; 3.3303x over previous
"""Trainium2 Bass kernel for the BERT span-pair classifier problem.

Computes, for B=2 batches over a 252x252 span-pair grid:
    h    = relu(Ai[i] + Aj[j] + ind(i,j)*w1c + b1)        # [770] per pair
    out  = h @ W2.T + b2                                   # [36]  per pair
    out  = where(span_mask >= 1, out, 0)
    res  = log_softmax(out over the 63504 pairs)           # per (batch, label)
    return res transposed to [B, 36, L*L]

Strategy (8 NeuronCores, SPMD single program):
  - The 504 (batch, row-i) rows are distributed over 8 cores, grouped into 4
    slot segments (in-span b0, in-span b1, off-span b0, off-span b1) so that
    every core runs an identical instruction template per slot.  In-span
    segments are assigned round-robin (row = s_b + 8*k + core) so the only
    runtime-varying quantity (the span boundary j = i) is `static + core_id`.
  - h is produced in [hid-chunk(128) x j] layout by fused tensor_scalar
    (add per-row bias, relu) ops; the span indicator is realized by a second
    windowed overwrite sourced from BjE = Bj0 + w1c * [j <= e].
  - The 770->36 matmul streams h as the moving operand against stationary
    W2^T chunks; mask, b2 and a -BIG invalid-pair offset are folded in as
    extra contraction rows.
  - exp + per-tile softmax sums come from ACT activation(Exp, accum_out=..);
    the global log-sum-exp is combined with a tiny [36,4] AllReduce.
"""

import math
import os
from contextlib import ExitStack

import numpy as np

import concourse.bass as bass
import concourse.bacc as bacc
import concourse.tile as tile
from concourse import mybir
from concourse._compat import with_exitstack
from concourse.bass_utils import run_bass_kernel_spmd

L = 252
HID = 768
MLP = 770
NLAB = 36
B = 2
NC = 8
KC = 6           # full 128-row hid chunks (6*128 = 768)
BIGNEG = -30.0   # makes exp(invalid pair) ~ 0 while staying fp32-exact

FP32 = mybir.dt.float32
BF16 = mybir.dt.bfloat16
AF = mybir.ActivationFunctionType
ALU = mybir.AluOpType


def _ceil2(x):
    return x + (x & 1)


def plan_slots(spans):
    """Compute the slot layout shared by host + device.

    Returns dict with per-segment (kind, batch, start_slot, nslots, rows).
    In-span segments: core c, slot p holds global row  s_b + 8*(p-start) + c.
    Off-span segments: arbitrary round-robin over the listed rows.
    """
    segs = []
    slot = 0
    for b in range(B):
        s, e = spans[b]
        n = e - s + 1
        nsl = _ceil2(math.ceil(n / NC))
        segs.append(dict(kind="in", b=b, start=slot, nslots=nsl, s=s, e=e, count=n))
        slot += nsl
    for b in range(B):
        s, e = spans[b]
        rows = [r for r in range(L) if r < s or r > e]
        nsl = _ceil2(math.ceil(len(rows) / NC))
        segs.append(dict(kind="off", b=b, start=slot, nslots=nsl, rows=rows,
                         count=len(rows)))
        slot += nsl
    nslot = slot
    assert nslot % 2 == 0
    return segs, nslot


def slot_map_for_core(segs, nslot, c):
    """-> list over slots of (batch, global_row) or None for padding."""
    m = [None] * nslot
    for sg in segs:
        for k in range(sg["nslots"]):
            idx = NC * k + c
            p = sg["start"] + k
            if idx < sg["count"]:
                if sg["kind"] == "in":
                    m[p] = (sg["b"], sg["s"] + idx)
                else:
                    m[p] = (sg["b"], sg["rows"][idx])
    return m


def build_kernel(spans, segs, nslot, engine_plan):
    """Returns a tile-kernel closure over the compile-time span values."""
    ntile = nslot // 2
    # batch of each slot (static, same on every core)
    slot_batch = []
    slot_seg = []
    for sg in segs:
        for k in range(sg["nslots"]):
            slot_batch.append(sg["b"])
            slot_seg.append(sg)
    # tiles owned by each batch (for the LSE reduction) -- contiguous runs
    btiles = {b: [t for t in range(ntile)
                  if slot_batch[2 * t] == b] for b in range(B)}
    for b in range(B):
        ts_ = btiles[b]
        assert all(slot_batch[2 * t] == slot_batch[2 * t + 1] for t in ts_)

    HW = 1024          # per-slot h scratch width (252 real + padding)
    SLOTW = 512        # per-slot region inside an h tile

    @with_exitstack
    def kern(ctx: ExitStack, tc: tile.TileContext, outs, ins):
        nc = tc.nc
        w1iT = ins["w1iT"]      # [768, 770] bf16 (DRAM)
        w1jT = ins["w1jT"]      # [768, 770] bf16
        w1c = ins["w1c"]        # [770] f32
        b1 = ins["b1"]          # [770] f32
        w2T6 = ins["w2T6"]      # [768, 36] bf16  (chunks 0..5 of W2^T)
        w2Tt = ins["w2Tt"]      # [4, 36] bf16    (rows 768,769, b2, ones*BIGNEG)
        vecsT = ins["vecsT"]    # [768, 504] bf16  (both batches, all j)
        myvT = ins["myvT"]      # [768, nslot] bf16 (per-core slot rows)
        maskb = ins["maskb"]    # [2, nslot*512] bf16: row0 = m, row1 = 1-m
        maskf = ins["maskf"]    # [nslot, 252] f32 (0/1 per-slot mask rows)
        flags = ins["flags"]    # [nslot, 2] f32: col b = 1 if slot is real row of b
        e2f = ins["e2f"]        # [128, 2] f32: col b = 1 if this core owns row s_b
        outd = outs["out"]      # [36, nslot*252] f32
        lseo = outs["lse"]      # [36, 4] f32 (debug: partial/total sums)

        fp = ctx.enter_context(tc.tile_pool(name="fp", bufs=1))
        prep_ps = ctx.enter_context(tc.tile_pool(name="prep_ps", bufs=1, space="PSUM"))
        main_ps = ctx.enter_context(tc.tile_pool(name="main_ps", bufs=4, space="PSUM"))
        cnt_ps = ctx.enter_context(tc.tile_pool(name="cnt_ps", bufs=1, space="PSUM"))
        hp = [ctx.enter_context(tc.tile_pool(name=f"h{c}", bufs=3)) for c in range(7)]
        mrp = ctx.enter_context(tc.tile_pool(name="mrp", bufs=3))
        esp = ctx.enter_context(tc.tile_pool(name="esp", bufs=2))
        dram = ctx.enter_context(tc.tile_pool(name="dram", bufs=1, space="DRAM"))

        # ---- persistent SBUF ----
        s_w1i = [fp.tile([128, MLP], BF16, tag=f"w1i{k}", name=f"w1i{k}") for k in range(KC)]
        s_w1j = [fp.tile([128, MLP], BF16, tag=f"w1j{k}", name=f"w1j{k}") for k in range(KC)]
        s_vT = [fp.tile([128, 2 * L], BF16, tag=f"vT{k}", name=f"vT{k}") for k in range(KC)]
        s_myv = [fp.tile([128, nslot], BF16, tag=f"myv{k}", name=f"myv{k}") for k in range(KC)]
        s_w2 = fp.tile([128, 6 * NLAB], BF16)      # W2T chunks 0..5 side by side
        s_w2h = fp.tile([2, NLAB], BF16)           # W2T rows 768,769
        s_w2m = fp.tile([2, NLAB], BF16)           # [b2 ; BIGNEG] rows
        s_w1c = fp.tile([128, 7], FP32)            # w1c per-chunk columns
        s_b1 = fp.tile([128, 7], FP32)
        s_b1w = fp.tile([128, 7], FP32)
        s_e2f = fp.tile([128, 2], FP32)
        CS = [128] * KC + [2]                      # chunk sizes of 770
        # Bj0 (bf16, A-source), BjE (f32, B-source), per batch padded to 512
        s_bj0 = [fp.tile([128, B * 512], BF16, tag=f"bj0_{c}", name=f"bj0_{c}") for c in range(7)]
        s_bje = [fp.tile([128, B * 512], FP32, tag=f"bje_{c}", name=f"bje_{c}") for c in range(7)]
        s_ai = [fp.tile([128, nslot], FP32, tag=f"ai{c}", name=f"ai{c}") for c in range(7)]
        s_aiE2 = [fp.tile([128, B], FP32, tag=f"aiE2{c}", name=f"aiE2{c}") for c in range(7)]
        s_out = fp.tile([NLAB, nslot * L], FP32)
        s_acc = fp.tile([NLAB, ntile], FP32)
        s_mrow = fp.tile([nslot, L], FP32)
        s_flag = fp.tile([nslot, 2], FP32)
        s_ones = fp.tile([nslot, NLAB], BF16)
        s_sums = fp.tile([NLAB, 4], FP32)          # [cnt_b0,cnt_b1,sum_b0,sum_b1]
        s_nlse = fp.tile([NLAB, B], FP32)

        # ---- load constants ----
        for k in range(KC):
            nc.sync.dma_start(out=s_w1i[k], in_=w1iT[128 * k:128 * (k + 1), :])
            nc.sync.dma_start(out=s_w1j[k], in_=w1jT[128 * k:128 * (k + 1), :])
            nc.sync.dma_start(out=s_vT[k], in_=vecsT[128 * k:128 * (k + 1), :])
            nc.sync.dma_start(out=s_myv[k], in_=myvT[128 * k:128 * (k + 1), :])
            nc.sync.dma_start(out=s_w2[:, NLAB * k:NLAB * (k + 1)],
                              in_=w2T6[128 * k:128 * (k + 1), :])
        nc.sync.dma_start(out=s_w2h, in_=w2Tt[0:2, :])
        nc.sync.dma_start(out=s_w2m, in_=w2Tt[2:4, :])
        # w1c/b1 [770] -> [128, 7] (partition p, col c) = v[128*c + p]; the
        # last column only has 2 valid rows -- load cols 0..5 strided + tail.
        nc.vector.memset(s_w1c, 0.0)
        nc.vector.memset(s_b1, 0.0)
        w1c2 = w1c[0:HID].rearrange("(c p) -> p c", p=128)
        nc.sync.dma_start(out=s_w1c[:, 0:KC], in_=w1c2)
        nc.sync.dma_start(out=s_w1c[0:2, KC:7], in_=w1c[HID:MLP].rearrange("(p o) -> p o", o=1))
        b12 = b1[0:HID].rearrange("(c p) -> p c", p=128)
        nc.sync.dma_start(out=s_b1[:, 0:KC], in_=b12)
        nc.sync.dma_start(out=s_b1[0:2, KC:7], in_=b1[HID:MLP].rearrange("(p o) -> p o", o=1))
        nc.sync.dma_start(out=s_e2f, in_=e2f)
        nc.vector.tensor_tensor(out=s_b1w, in0=s_b1, in1=s_w1c, op=ALU.add)
        nc.sync.dma_start(out=s_mrow, in_=maskf)
        nc.sync.dma_start(out=s_flag, in_=flags)
        nc.vector.memset(s_ones, 1.0)

        # ---- prep: AjT -> Bj0/BjE, AiT ----
        for c in range(7):
            cs = CS[c]
            mlo = 128 * c
            # Bj (= Aj + b1) for all 504 (b, j) columns
            psA = prep_ps.tile([128, 2 * L], FP32, tag="psA", name=f"psA{c}")
            for k in range(KC):
                nc.tensor.matmul(psA[:cs, :], s_w1j[k][:, mlo:mlo + cs], s_vT[k],
                                 start=(k == 0), stop=(k == KC - 1))
            bje = s_bje[c]
            for b in range(B):
                e = spans[b][1]
                # Bj0 (bf16, A-source) = Aj + b1
                nc.vector.tensor_scalar(
                    out=s_bj0[c][:cs, 512 * b:512 * b + L],
                    in0=psA[:cs, L * b:L * (b + 1)],
                    scalar1=s_b1[:cs, c:c + 1], scalar2=None, op0=ALU.add)
                nc.gpsimd.memset(s_bj0[c][:cs, 512 * b + L:512 * (b + 1)], 0.0)
                # BjE (f32, B-source) = Aj + b1 + w1c*[j<=e]
                nc.vector.tensor_scalar(
                    out=bje[:cs, 512 * b:512 * b + e + 1],
                    in0=psA[:cs, L * b:L * b + e + 1],
                    scalar1=s_b1w[:cs, c:c + 1], scalar2=None, op0=ALU.add)
                if e + 1 < L:
                    nc.vector.tensor_scalar(
                        out=bje[:cs, 512 * b + e + 1:512 * b + L],
                        in0=psA[:cs, L * b + e + 1:L * (b + 1)],
                        scalar1=s_b1[:cs, c:c + 1], scalar2=None, op0=ALU.add)
                nc.vector.memset(bje[:cs, 512 * b + L:512 * (b + 1)], 0.0)
            # AiT for my slots
            psI = prep_ps.tile([128, nslot], FP32, tag="psI", name=f"psI{c}")
            for k in range(KC):
                nc.tensor.matmul(psI[:cs, :], s_w1i[k][:, mlo:mlo + cs], s_myv[k],
                                 start=(k == 0), stop=(k == KC - 1))
            nc.vector.tensor_copy(out=s_ai[c][:cs, :], in_=psI[:cs, :])
            # E2 bias: Ai[slot_e2] + w1c * own_flag  (per batch)
            for b in range(B):
                p_e2 = next(sg for sg in segs
                            if sg["kind"] == "in" and sg["b"] == b)["start"]
                nc.vector.tensor_tensor(
                    out=s_aiE2[c][:cs, b:b + 1],
                    in0=s_w1c[:cs, c:c + 1], in1=s_e2f[:cs, b:b + 1],
                    op=ALU.mult)
                nc.vector.tensor_tensor(
                    out=s_aiE2[c][:cs, b:b + 1],
                    in0=s_aiE2[c][:cs, b:b + 1], in1=s_ai[c][:cs, p_e2:p_e2 + 1],
                    op=ALU.add)

        pid = {}

        def eng_pid(eng):
            if eng not in pid:
                pid[eng] = eng.partition_id()
            return pid[eng]

        def ts_relu(eng, out, in0, sc):
            if eng is nc.scalar:
                nc.scalar.activation(out, in0, AF.Relu, bias=sc, scale=1.0)
            else:
                eng.tensor_scalar(out=out, in0=in0, scalar1=sc, scalar2=0.0,
                                  op0=ALU.add, op1=ALU.max)

        # ---- main loop over 2-slot tiles ----
        for t in range(ntile):
            hts = []
            for c in range(7):
                cs = CS[c]
                ht = hp[c].tile([cs, HW], BF16, tag=f"ht{c}", name=f"ht{c}_{t}")
                hts.append(ht)
                for sl in range(2):
                    p = 2 * t + sl
                    b = slot_batch[p]
                    sg = slot_seg[p]
                    base = SLOTW * sl
                    engA, engB = engine_plan(t, c, p)
                    # segment A: whole row from Bj0 (correct outside [i, e])
                    ts_relu(engA, ht[:cs, base:base + 256],
                            s_bj0[c][:cs, 512 * b:512 * b + 256],
                            s_ai[c][:cs, p:p + 1])
                    if sg["kind"] == "in":
                        # segment B: overwrite [i, i+256) from BjE (adds w1c
                        # on [i, e]; equals Bj0 past e)
                        kk = p - sg["start"]
                        ioff = sg["s"] + NC * kk + eng_pid(engB)
                        ts_relu(engB, ht[:cs, bass.ds(base + ioff, 256)],
                                s_bje[c][:cs, bass.ds(512 * b + ioff, 256)],
                                s_ai[c][:cs, p:p + 1])
                    # E2 pixel: slot holding row s_b gets +2*w1c at j=e_b
                    if sg["kind"] == "in" and p == sg["start"]:
                        e = sg["e"]
                        ts_relu(nc.vector, ht[:cs, base + e:base + e + 1],
                                s_bje[c][:cs, 512 * b + e:512 * b + e + 1],
                                s_aiE2[c][:cs, b:b + 1])

            # matmul: psum[36, 504] over 6 full chunks + tail + mask rows
            ps = main_ps.tile([NLAB, 2 * L], FP32, tag="ps", name=f"ps{t}")
            rhs6 = [hts[c][:, :].rearrange("p (s w) -> p s w", w=SLOTW)
                    [:, :, 0:L] for c in range(6)]
            for c in range(6):
                nc.tensor.matmul(ps, s_w2[:, NLAB * c:NLAB * (c + 1)], rhs6[c],
                                 start=(c == 0), stop=False)
            rhs_t = hts[6][:, :].rearrange("p (s w) -> p s w", w=SLOTW)[:, :, 0:L]
            nc.tensor.matmul(ps, s_w2h, rhs_t, start=False, stop=False)
            mb = maskb[:, HW * t:HW * (t + 1)].rearrange("p (s w) -> p s w", w=SLOTW)
            mrhs = mrp.tile([2, HW], BF16, tag="mb", name=f"mb{t}")
            nc.sync.dma_start(out=mrhs, in_=maskb[:, HW * t:HW * (t + 1)])
            nc.tensor.matmul(
                ps, s_w2m,
                mrhs[:, :].rearrange("p (s w) -> p s w", w=SLOTW)[:, :, 0:L],
                start=False, stop=True)

            # exp + per-tile softmax sum (invalid pairs contribute ~0)
            esc = esp.tile([NLAB, 2 * L], FP32, tag="esc", name=f"esc{t}")
            nc.scalar.activation(esc, ps, AF.Exp, accum_out=s_acc[:, t:t + 1])

            # masked values -> out buffer (mask replicated over 36 partitions)
            mrep = mrp.tile([NLAB, 2 * L], FP32, tag="mrep", name=f"mrep{t}")
            msrc = maskf[2 * t:2 * t + 2, :]
            nc.sync.dma_start(
                out=mrep,
                in_=bass.AP(tensor=msrc.tensor, offset=msrc.offset,
                            ap=[[0, NLAB], [L, 2], [1, L]]))
            nc.vector.tensor_tensor(out=s_out[:, 2 * L * t:2 * L * (t + 1)],
                                    in0=ps, in1=mrep, op=ALU.mult)

        # ---- LSE: counts + tile sums + AllReduce + log ----
        srow = fp.tile([nslot, 1], FP32)
        nc.vector.tensor_reduce(out=srow, in_=s_mrow, axis=mybir.AxisListType.X, op=ALU.add)
        vb = fp.tile([nslot, B], BF16)
        vf = fp.tile([nslot, B], FP32)
        for b in range(B):
            nc.vector.tensor_scalar(out=vf[:, b:b + 1], in0=srow,
                                    scalar1=-1.0, scalar2=float(L),
                                    op0=ALU.mult, op1=ALU.add)
        nc.vector.tensor_tensor(out=vb, in0=vf, in1=s_flag, op=ALU.mult)
        cps = cnt_ps.tile([NLAB, B], FP32, tag="cnt", name="cnt")
        nc.tensor.matmul(cps, s_ones[:, 0:NLAB], vb, start=True, stop=True)
        nc.vector.tensor_copy(out=s_sums[:, 0:B], in_=cps)
        for b in range(B):
            rs = btiles[b]
            # contiguous runs of tiles for this batch
            runs = []
            st = rs[0]
            for a, bb in zip(rs, rs[1:] + [None]):
                if bb != a + 1:
                    runs.append((st, a))
                    st = bb
            t0, t1 = runs[0]
            nc.vector.tensor_reduce(out=s_sums[:, 2 + b:3 + b],
                                    in_=s_acc[:, t0:t1 + 1],
                                    axis=mybir.AxisListType.X, op=ALU.add)
            for (u0, u1) in runs[1:]:
                tmp = fp.tile([NLAB, 1], FP32, tag=f"tr{b}", name=f"tr{b}_{u0}")
                nc.vector.tensor_reduce(out=tmp, in_=s_acc[:, u0:u1 + 1],
                                        axis=mybir.AxisListType.X, op=ALU.add)
                nc.vector.tensor_tensor(out=s_sums[:, 2 + b:3 + b],
                                        in0=s_sums[:, 2 + b:3 + b], in1=tmp,
                                        op=ALU.add)
        # partial = tilesum + invalid count
        for b in range(B):
            nc.vector.tensor_tensor(out=s_sums[:, 2 + b:3 + b],
                                    in0=s_sums[:, 2 + b:3 + b],
                                    in1=s_sums[:, b:b + 1], op=ALU.add)
            nc.vector.memset(s_sums[:, b:b + 1], 0.0)
        cc_in = dram.tile([NLAB, 4], FP32, name="cc_in")
        cc_out = nc.dram_tensor("cc_out", [NLAB, 4], FP32, kind="Internal",
                                addr_space="Shared").ap()
        nc.sync.dma_start(out=cc_in, in_=s_sums)
        nc.gpsimd.collective_compute(
            "AllReduce", ALU.add, replica_groups=[list(range(NC))],
            ins=[cc_in[:]], outs=[cc_out], cc_dim="Partition")
        allsum = fp.tile([NLAB, 4], FP32)
        nc.sync.dma_start(out=allsum, in_=cc_out)
        nc.sync.dma_start(out=lseo, in_=allsum)
        nc.scalar.activation(s_nlse, allsum[:, 2:4], AF.Ln)
        nc.vector.tensor_scalar(out=s_nlse, in0=s_nlse, scalar1=-1.0,
                                scalar2=None, op0=ALU.mult)

        # ---- final: out - LSE, store ----
        for t in range(ntile):
            b = slot_batch[2 * t]
            seg = s_out[:, 2 * L * t:2 * L * (t + 1)]
            nc.vector.tensor_scalar(out=seg, in0=seg,
                                    scalar1=s_nlse[:, b:b + 1], scalar2=None,
                                    op0=ALU.add)
            nc.sync.dma_start(out=outd[:, 2 * L * t:2 * L * (t + 1)], in_=seg)

    return kern


def default_engine_plan(nc_getter):
    """A (static APs) on DVE; B (register-offset APs) on ACT -- ScalarE has
    the fast scalar_dynamic_offset path, VectorE dynamic APs hit a ~4us
    fallback, and GPSIMD tensor_scalar measures ~15x slower than DVE."""
    def plan(t, c, p):
        nc = nc_getter()
        return nc.vector, nc.scalar
    return plan


def kernel(**inputs) -> np.ndarray:
    hidden = np.asarray(inputs["hidden"], dtype=np.float32)
    pred_spans = np.asarray(inputs["pred_spans"]).astype(np.int64)
    span_mask = np.asarray(inputs["span_mask"]).astype(np.int32)
    W1 = np.asarray(inputs["W1"], dtype=np.float32)
    b1 = np.asarray(inputs["b1"], dtype=np.float32)
    W2 = np.asarray(inputs["W2"], dtype=np.float32)
    b2 = np.asarray(inputs["b2"], dtype=np.float32)

    spans = [(int(pred_spans[b, 0]), int(pred_spans[b, 1])) for b in range(B)]
    segs, nslot = plan_slots(spans)
    ntile = nslot // 2

    vecs = hidden[:, 1:L + 1, :]                       # [B, L, 768]
    vecsT = np.concatenate([vecs[0].T, vecs[1].T], axis=1)   # [768, 504]
    W1T = W1.T                                          # [1537, 770]
    w1iT = np.ascontiguousarray(W1T[0:HID]).astype(np.float32)
    w1jT = np.ascontiguousarray(W1T[HID:2 * HID]).astype(np.float32)
    w1c = np.ascontiguousarray(W1T[2 * HID]).astype(np.float32)
    W2T = np.ascontiguousarray(W2.T)                    # [770, 36]
    w2T6 = W2T[0:HID]
    w2Tt = np.stack([W2T[768], W2T[769], b2,
                     np.full(NLAB, BIGNEG, np.float32)], axis=0)

    maskf_full = span_mask.astype(np.float32).clip(0, 1)

    in_maps = []
    slot_maps = []
    for c in range(NC):
        sm = slot_map_for_core(segs, nslot, c)
        slot_maps.append(sm)
        myv = np.zeros((HID, nslot), np.float32)
        maskf = np.zeros((nslot, L), np.float32)
        flags = np.zeros((nslot, 2), np.float32)
        for p, ent in enumerate(sm):
            if ent is None:
                continue
            b, r = ent
            myv[:, p] = vecs[b, r]
            maskf[p] = maskf_full[r]
            flags[p, b] = 1.0
        maskb = np.zeros((2, nslot * 512), np.float32)
        for t in range(ntile):
            for sl in range(2):
                p = 2 * t + sl
                maskb[0, 1024 * t + 512 * sl:1024 * t + 512 * sl + L] = maskf[p]
                maskb[1, 1024 * t + 512 * sl:1024 * t + 512 * sl + L] = \
                    1.0 - maskf[p]
        e2f = np.zeros((128, 2), np.float32)
        for b in range(B):
            if c == 0:
                e2f[:, b] = 1.0      # row s_b lives on core 0 (slot seg start)
        in_maps.append({
            "w1iT": w1iT, "w1jT": w1jT, "w1c": w1c, "b1": b1,
            "w2T6": w2T6, "w2Tt": w2Tt,
            "vecsT": vecsT, "myvT": myv,
            "maskb": maskb, "maskf": maskf, "flags": flags, "e2f": e2f,
        })

    # ---- build program ----
    nc = bacc.Bacc("TRN2", target_bir_lowering=False, debug=False,
                   enable_asserts=False, num_devices=NC)

    def mk(name, arr, dt):
        return nc.dram_tensor(name, list(arr.shape), dt, kind="ExternalInput").ap()

    ml_bf = lambda n, a: mk(n, a, BF16)
    ml_f32 = lambda n, a: mk(n, a, FP32)
    ex = in_maps[0]
    ins_aps = {
        "w1iT": ml_bf("w1iT", ex["w1iT"]), "w1jT": ml_bf("w1jT", ex["w1jT"]),
        "w1c": ml_f32("w1c", ex["w1c"]), "b1": ml_f32("b1", ex["b1"]),
        "w2T6": ml_bf("w2T6", ex["w2T6"]), "w2Tt": ml_bf("w2Tt", ex["w2Tt"]),
        "vecsT": ml_bf("vecsT", ex["vecsT"]), "myvT": ml_bf("myvT", ex["myvT"]),
        "maskb": ml_bf("maskb", ex["maskb"]), "maskf": ml_f32("maskf", ex["maskf"]),
        "flags": ml_f32("flags", ex["flags"]), "e2f": ml_f32("e2f", ex["e2f"]),
    }
    outs_aps = {
        "out": nc.dram_tensor("out", [NLAB, nslot * L], FP32,
                              kind="ExternalOutput").ap(),
        "lse": nc.dram_tensor("lse", [NLAB, 4], FP32,
                              kind="ExternalOutput").ap(),
    }

    plan = default_engine_plan(lambda: nc)
    kern = build_kernel(spans, segs, nslot, plan)
    with tile.TileContext(nc) as t:
        kern(t, outs_aps, ins_aps)
    nc.compile()

    # bf16-cast the bf16 inputs host-side
    def cast_maps(m):
        out = {}
        for k, v in m.items():
            dt = ins_aps[k].dtype
            if dt == BF16:
                out[k] = v.astype(mybir.dt.np(BF16))
            else:
                out[k] = v.astype(np.float32)
        return out

    in_maps_c = [cast_maps(m) for m in in_maps]

    if os.environ.get("BK_BUILD_ONLY"):
        print("BUILD OK")
        return np.zeros((B, NLAB, L * L), np.float32)

    if os.environ.get("BK_SIM"):
        from concourse.bass_interp import MultiCoreSim

        sim = MultiCoreSim(nc, num_cores=NC, require_finite=False,
                           require_nnan=False)
        for c, cs in sim.cores.items():
            for name, arr in in_maps_c[c].items():
                cs.tensor(name)[:] = arr
            if nc.partition_id_tensor is not None:
                cs.tensor(nc.partition_id_tensor.name)[:] = np.array(
                    [[c]], dtype=np.uint32)
        sim.simulate(check_with_hw=False)

        class _R:
            results = [{"out": np.asarray(sim.cores[c].tensor("out")),
                        "lse": np.asarray(sim.cores[c].tensor("lse"))}
                       for c in range(NC)]
        res = _R()
    else:
        trace = bool(int(os.environ.get("BK_TRACE", "0")))
        res = run_bass_kernel_spmd(nc, in_maps_c, core_ids=list(range(NC)),
                                   trace=trace)
        if trace and res.exec_time_ns is not None:
            print(f"HW exec time: {res.exec_time_ns} ns")

    # ---- unshard ----
    out_full = np.zeros((B, NLAB, L * L), np.float32)
    for c in range(NC):
        oc = res.results[c]["out"]          # [36, nslot*252]
        for p, ent in enumerate(slot_maps[c]):
            if ent is None:
                continue
            b, r = ent
            out_full[b, :, L * r:L * (r + 1)] = oc[:, L * p:L * (p + 1)]
    return out_full


# revision 15
# speedup vs baseline: 3.5905x; 1.0781x over previous
"""Trainium2 Bass kernel for the BERT span-pair classifier problem.

Computes, for B=2 batches over a 252x252 span-pair grid:
    h    = relu(Ai[i] + Aj[j] + ind(i,j)*w1c + b1)        # [770] per pair
    out  = h @ W2.T + b2                                   # [36]  per pair
    out  = where(span_mask >= 1, out, 0)
    res  = log_softmax(out over the 63504 pairs)           # per (batch, label)
    return res transposed to [B, 36, L*L]

Strategy (8 NeuronCores, SPMD single program):
  - The 504 (batch, row-i) rows are distributed over 8 cores, grouped into 4
    slot segments (in-span b0, in-span b1, off-span b0, off-span b1) so that
    every core runs an identical instruction template per slot.  In-span
    segments are assigned round-robin (row = s_b + 8*k + core) so the only
    runtime-varying quantity (the span boundary j = i) is `static + core_id`.
  - h is produced in [hid-chunk(128) x j] layout by fused tensor_scalar
    (add per-row bias, relu) ops; the span indicator is realized by a second
    windowed overwrite sourced from BjE = Bj0 + w1c * [j <= e].
  - The 770->36 matmul streams h as the moving operand against stationary
    W2^T chunks; mask, b2 and a -BIG invalid-pair offset are folded in as
    extra contraction rows.
  - exp + per-tile softmax sums come from ACT activation(Exp, accum_out=..);
    the global log-sum-exp is combined with a tiny [36,4] AllReduce.
"""

import math
import os
from contextlib import ExitStack

import numpy as np

import concourse.bass as bass
import concourse.bacc as bacc
import concourse.tile as tile
from concourse import mybir
from concourse._compat import with_exitstack
from concourse.bass_utils import run_bass_kernel_spmd

L = 252
HID = 768
MLP = 770
NLAB = 36
B = 2
NC = 8
KC = 6           # full 128-row hid chunks (6*128 = 768)
BIGNEG = -30.0   # makes exp(invalid pair) ~ 0 while staying fp32-exact

FP32 = mybir.dt.float32
BF16 = mybir.dt.bfloat16
AF = mybir.ActivationFunctionType
ALU = mybir.AluOpType


def _ceil2(x):
    return x + (x & 1)


def plan_slots(spans):
    """Compute the slot layout shared by host + device.

    Returns dict with per-segment (kind, batch, start_slot, nslots, rows).
    In-span segments: core c, slot p holds global row  s_b + 8*(p-start) + c.
    Off-span segments: arbitrary round-robin over the listed rows.
    """
    segs = []
    slot = 0
    for b in range(B):
        s, e = spans[b]
        n = e - s + 1
        nsl = _ceil2(math.ceil(n / NC))
        segs.append(dict(kind="in", b=b, start=slot, nslots=nsl, s=s, e=e, count=n))
        slot += nsl
    for b in range(B):
        s, e = spans[b]
        rows = [r for r in range(L) if r < s or r > e]
        nsl = _ceil2(math.ceil(len(rows) / NC))
        segs.append(dict(kind="off", b=b, start=slot, nslots=nsl, rows=rows,
                         count=len(rows)))
        slot += nsl
    nslot = slot
    assert nslot % 2 == 0
    return segs, nslot


def slot_map_for_core(segs, nslot, c):
    """-> list over slots of (batch, global_row) or None for padding."""
    m = [None] * nslot
    for sg in segs:
        for k in range(sg["nslots"]):
            idx = NC * k + c
            p = sg["start"] + k
            if idx < sg["count"]:
                if sg["kind"] == "in":
                    m[p] = (sg["b"], sg["s"] + idx)
                else:
                    m[p] = (sg["b"], sg["rows"][idx])
    return m


def build_kernel(spans, segs, nslot, engine_plan):
    """Returns a tile-kernel closure over the compile-time span values."""
    ntile = nslot // 2
    # batch of each slot (static, same on every core)
    slot_batch = []
    slot_seg = []
    for sg in segs:
        for k in range(sg["nslots"]):
            slot_batch.append(sg["b"])
            slot_seg.append(sg)
    # tiles owned by each batch (for the LSE reduction) -- contiguous runs
    btiles = {b: [t for t in range(ntile)
                  if slot_batch[2 * t] == b] for b in range(B)}
    for b in range(B):
        ts_ = btiles[b]
        assert all(slot_batch[2 * t] == slot_batch[2 * t + 1] for t in ts_)

    HW = 1024          # per-slot h scratch width (252 real + padding)
    SLOTW = 512        # per-slot region inside an h tile

    @with_exitstack
    def kern(ctx: ExitStack, tc: tile.TileContext, outs, ins):
        nc = tc.nc
        w1iT = ins["w1iT"]      # [768, 770] bf16 (DRAM)
        w1jT = ins["w1jT"]      # [768, 770] bf16
        w1c = ins["w1c"]        # [770] f32
        b1 = ins["b1"]          # [770] f32
        w2T6 = ins["w2T6"]      # [768, 36] bf16  (chunks 0..5 of W2^T)
        w2Tt = ins["w2Tt"]      # [4, 36] bf16    (rows 768,769, b2, ones*BIGNEG)
        vecsT = ins["vecsT"]    # [768, 504] bf16  (both batches, all j)
        myvT = ins["myvT"]      # [768, nslot] bf16 (per-core slot rows)
        maskb = ins["maskb"]    # [2, nslot*512] bf16: row0 = m, row1 = 1-m
        maskf = ins["maskf"]    # [nslot, 252] f32 (0/1 per-slot mask rows)
        flags = ins["flags"]    # [nslot, 2] f32: col b = 1 if slot is real row of b
        e2f = ins["e2f"]        # [128, 2] f32: col b = 1 if this core owns row s_b
        outd = outs["out"]      # [36, nslot*252] f32
        lseo = outs["lse"]      # [36, 4] f32 (debug: partial/total sums)

        fp = ctx.enter_context(tc.tile_pool(name="fp", bufs=1))
        prep_ps = ctx.enter_context(tc.tile_pool(name="prep_ps", bufs=1, space="PSUM"))
        main_ps = ctx.enter_context(tc.tile_pool(name="main_ps", bufs=4, space="PSUM"))
        cnt_ps = ctx.enter_context(tc.tile_pool(name="cnt_ps", bufs=1, space="PSUM"))
        hp = [ctx.enter_context(tc.tile_pool(name=f"h{c}", bufs=4)) for c in range(7)]
        mrp = ctx.enter_context(tc.tile_pool(name="mrp", bufs=3))
        esp = ctx.enter_context(tc.tile_pool(name="esp", bufs=2))
        dram = ctx.enter_context(tc.tile_pool(name="dram", bufs=1, space="DRAM"))

        # ---- persistent SBUF ----
        s_w1i = [fp.tile([128, MLP], BF16, tag=f"w1i{k}", name=f"w1i{k}") for k in range(KC)]
        s_w1j = [fp.tile([128, MLP], BF16, tag=f"w1j{k}", name=f"w1j{k}") for k in range(KC)]
        s_vT = [fp.tile([128, 2 * L], BF16, tag=f"vT{k}", name=f"vT{k}") for k in range(KC)]
        s_myv = [fp.tile([128, nslot], BF16, tag=f"myv{k}", name=f"myv{k}") for k in range(KC)]
        s_w2 = fp.tile([128, 6 * NLAB], BF16)      # W2T chunks 0..5 side by side
        s_w2t4 = fp.tile([4, NLAB], BF16)          # [W2T768, W2T769, b2, BIGNEG]
        s_w1c = fp.tile([128, 7], FP32)            # w1c per-chunk columns
        s_b1 = fp.tile([128, 7], FP32)
        s_b1w = fp.tile([128, 7], FP32)
        s_e2f = fp.tile([128, 2], FP32)
        CS = [128] * KC + [2]                      # chunk sizes of 770
        # Bj0 (bf16, A-source), BjE (f32, B-source), per batch padded to 512
        s_bj0 = [fp.tile([128, B * 512], BF16, tag=f"bj0_{c}", name=f"bj0_{c}") for c in range(7)]
        s_bje = [fp.tile([128, B * 512], FP32, tag=f"bje_{c}", name=f"bje_{c}") for c in range(7)]
        s_ai = [fp.tile([128, nslot], FP32, tag=f"ai{c}", name=f"ai{c}") for c in range(7)]
        s_aiE2 = [fp.tile([128, B], FP32, tag=f"aiE2{c}", name=f"aiE2{c}") for c in range(7)]
        s_out = fp.tile([NLAB, nslot * L], FP32)
        s_acc = fp.tile([NLAB, ntile], FP32)
        s_mrow = fp.tile([nslot, L], FP32)
        s_flag = fp.tile([nslot, 2], FP32)
        s_ones = fp.tile([nslot, NLAB], BF16)
        s_sums = fp.tile([NLAB, 4], FP32)          # [cnt_b0,cnt_b1,sum_b0,sum_b1]
        s_nlse = fp.tile([NLAB, B], FP32)

        # ---- load constants ----
        for k in range(KC):
            nc.sync.dma_start(out=s_w1i[k], in_=w1iT[128 * k:128 * (k + 1), :])
            nc.sync.dma_start(out=s_w1j[k], in_=w1jT[128 * k:128 * (k + 1), :])
            nc.sync.dma_start(out=s_vT[k], in_=vecsT[128 * k:128 * (k + 1), :])
            nc.sync.dma_start(out=s_myv[k], in_=myvT[128 * k:128 * (k + 1), :])
            nc.sync.dma_start(out=s_w2[:, NLAB * k:NLAB * (k + 1)],
                              in_=w2T6[128 * k:128 * (k + 1), :])
        nc.sync.dma_start(out=s_w2t4, in_=w2Tt)
        # w1c/b1 [770] -> [128, 7] (partition p, col c) = v[128*c + p]; the
        # last column only has 2 valid rows -- load cols 0..5 strided + tail.
        nc.vector.memset(s_w1c, 0.0)
        nc.vector.memset(s_b1, 0.0)
        w1c2 = w1c[0:HID].rearrange("(c p) -> p c", p=128)
        nc.sync.dma_start(out=s_w1c[:, 0:KC], in_=w1c2)
        nc.sync.dma_start(out=s_w1c[0:2, KC:7], in_=w1c[HID:MLP].rearrange("(p o) -> p o", o=1))
        b12 = b1[0:HID].rearrange("(c p) -> p c", p=128)
        nc.sync.dma_start(out=s_b1[:, 0:KC], in_=b12)
        nc.sync.dma_start(out=s_b1[0:2, KC:7], in_=b1[HID:MLP].rearrange("(p o) -> p o", o=1))
        nc.sync.dma_start(out=s_e2f, in_=e2f)
        nc.vector.tensor_tensor(out=s_b1w, in0=s_b1, in1=s_w1c, op=ALU.add)
        nc.sync.dma_start(out=s_mrow, in_=maskf)
        nc.sync.dma_start(out=s_flag, in_=flags)
        nc.vector.memset(s_ones, 1.0)

        # ---- prep: AjT -> Bj0/BjE, AiT ----
        for c in range(7):
            cs = CS[c]
            mlo = 128 * c
            # Bj (= Aj + b1) for all 504 (b, j) columns
            psA = prep_ps.tile([128, 2 * L], FP32, tag="psA", name=f"psA{c}")
            for k in range(KC):
                nc.tensor.matmul(psA[:cs, :], s_w1j[k][:, mlo:mlo + cs], s_vT[k],
                                 start=(k == 0), stop=(k == KC - 1))
            bje = s_bje[c]
            for b in range(B):
                e = spans[b][1]
                # Bj0 (bf16, A-source) = Aj + b1
                nc.vector.tensor_scalar(
                    out=s_bj0[c][:cs, 512 * b:512 * b + L],
                    in0=psA[:cs, L * b:L * (b + 1)],
                    scalar1=s_b1[:cs, c:c + 1], scalar2=None, op0=ALU.add)
                nc.gpsimd.memset(s_bj0[c][:cs, 512 * b + L:512 * (b + 1)], 0.0)
                # BjE (f32, B-source) = Aj + b1 + w1c*[j<=e]
                nc.vector.tensor_scalar(
                    out=bje[:cs, 512 * b:512 * b + e + 1],
                    in0=psA[:cs, L * b:L * b + e + 1],
                    scalar1=s_b1w[:cs, c:c + 1], scalar2=None, op0=ALU.add)
                if e + 1 < L:
                    nc.vector.tensor_scalar(
                        out=bje[:cs, 512 * b + e + 1:512 * b + L],
                        in0=psA[:cs, L * b + e + 1:L * (b + 1)],
                        scalar1=s_b1[:cs, c:c + 1], scalar2=None, op0=ALU.add)
                nc.vector.memset(bje[:cs, 512 * b + L:512 * (b + 1)], 0.0)
            # AiT for my slots
            psI = prep_ps.tile([128, nslot], FP32, tag="psI", name=f"psI{c}")
            for k in range(KC):
                nc.tensor.matmul(psI[:cs, :], s_w1i[k][:, mlo:mlo + cs], s_myv[k],
                                 start=(k == 0), stop=(k == KC - 1))
            nc.vector.tensor_copy(out=s_ai[c][:cs, :], in_=psI[:cs, :])
            # E2 bias: Ai[slot_e2] + w1c * own_flag  (per batch)
            for b in range(B):
                p_e2 = next(sg for sg in segs
                            if sg["kind"] == "in" and sg["b"] == b)["start"]
                nc.vector.tensor_tensor(
                    out=s_aiE2[c][:cs, b:b + 1],
                    in0=s_w1c[:cs, c:c + 1], in1=s_e2f[:cs, b:b + 1],
                    op=ALU.mult)
                nc.vector.tensor_tensor(
                    out=s_aiE2[c][:cs, b:b + 1],
                    in0=s_aiE2[c][:cs, b:b + 1], in1=s_ai[c][:cs, p_e2:p_e2 + 1],
                    op=ALU.add)

        pid = {}

        def eng_pid(eng):
            if eng not in pid:
                pid[eng] = eng.partition_id()
            return pid[eng]

        def ts_relu(eng, out, in0, sc):
            if eng is nc.scalar:
                nc.scalar.activation(out, in0, AF.Relu, bias=sc, scale=1.0)
            else:
                eng.tensor_scalar(out=out, in0=in0, scalar1=sc, scalar2=0.0,
                                  op0=ALU.add, op1=ALU.max)

        # ---- main loop over 2-slot tiles ----
        for t in range(ntile):
            hts = []
            for c in range(7):
                cs = CS[c] if c < 6 else 4
                ht = hp[c].tile([cs, HW], BF16, tag=f"ht{c}", name=f"ht{c}_{t}")
                hts.append(ht)
            nc.sync.dma_start(out=hts[6][2:4, :], in_=maskb[:, HW * t:HW * (t + 1)])
            for c in range(7):
                cs = CS[c]
                ht = hts[c]
                for sl in range(2):
                    p = 2 * t + sl
                    b = slot_batch[p]
                    sg = slot_seg[p]
                    base = SLOTW * sl
                    engA, engB = engine_plan(t, c, p)
                    # segment A: whole row from Bj0 (correct outside [i, e])
                    ts_relu(engA, ht[:cs, base:base + 256],
                            s_bj0[c][:cs, 512 * b:512 * b + 256],
                            s_ai[c][:cs, p:p + 1])
                    if sg["kind"] == "in":
                        # segment B: overwrite [i, i+W) from BjE (adds w1c on
                        # [i, e]; equals Bj0 on (e, e+core]).  W covers the
                        # span exactly for the worst core offset.
                        kk = p - sg["start"]
                        W = max(1, sg["e"] - sg["s"] - NC * kk + 1)
                        ioff = sg["s"] + NC * kk + eng_pid(engB)
                        ts_relu(engB, ht[:cs, bass.ds(base + ioff, W)],
                                s_bje[c][:cs, bass.ds(512 * b + ioff, W)],
                                s_ai[c][:cs, p:p + 1])
                    # E2 pixel: slot holding row s_b gets +2*w1c at j=e_b
                    if sg["kind"] == "in" and p == sg["start"]:
                        e = sg["e"]
                        ts_relu(nc.vector, ht[:cs, base + e:base + e + 1],
                                s_bje[c][:cs, 512 * b + e:512 * b + e + 1],
                                s_aiE2[c][:cs, b:b + 1])

            # matmul: psum[36, 504] over 6 full chunks + tail + mask rows
            ps = main_ps.tile([NLAB, 2 * L], FP32, tag="ps", name=f"ps{t}")
            rhs6 = [hts[c][:, :].rearrange("p (s w) -> p s w", w=SLOTW)
                    [:, :, 0:L] for c in range(6)]
            for c in range(6):
                nc.tensor.matmul(ps, s_w2[:, NLAB * c:NLAB * (c + 1)], rhs6[c],
                                 start=(c == 0), stop=False)
            rhs_t = hts[6][:, :].rearrange("p (s w) -> p s w", w=SLOTW)[:, :, 0:L]
            nc.tensor.matmul(ps, s_w2t4, rhs_t, start=False, stop=True)

            # exp + per-tile softmax sum (invalid pairs contribute ~0)
            esc = esp.tile([NLAB, 2 * L], FP32, tag="esc", name=f"esc{t}")
            nc.scalar.activation(esc, ps, AF.Exp, accum_out=s_acc[:, t:t + 1])

            # masked values -> out buffer (mask replicated over 36 partitions)
            mrep = mrp.tile([NLAB, 2 * L], FP32, tag="mrep", name=f"mrep{t}")
            msrc = maskf[2 * t:2 * t + 2, :]
            nc.sync.dma_start(
                out=mrep,
                in_=bass.AP(tensor=msrc.tensor, offset=msrc.offset,
                            ap=[[0, NLAB], [L, 2], [1, L]]))
            nc.vector.tensor_tensor(out=s_out[:, 2 * L * t:2 * L * (t + 1)],
                                    in0=ps, in1=mrep, op=ALU.mult)

        # ---- LSE: counts + tile sums + AllReduce + log ----
        srow = fp.tile([nslot, 1], FP32)
        nc.vector.tensor_reduce(out=srow, in_=s_mrow, axis=mybir.AxisListType.X, op=ALU.add)
        vb = fp.tile([nslot, B], BF16)
        vf = fp.tile([nslot, B], FP32)
        for b in range(B):
            nc.vector.tensor_scalar(out=vf[:, b:b + 1], in0=srow,
                                    scalar1=-1.0, scalar2=float(L),
                                    op0=ALU.mult, op1=ALU.add)
        nc.vector.tensor_tensor(out=vb, in0=vf, in1=s_flag, op=ALU.mult)
        cps = cnt_ps.tile([NLAB, B], FP32, tag="cnt", name="cnt")
        nc.tensor.matmul(cps, s_ones[:, 0:NLAB], vb, start=True, stop=True)
        nc.vector.tensor_copy(out=s_sums[:, 0:B], in_=cps)
        for b in range(B):
            rs = btiles[b]
            # contiguous runs of tiles for this batch
            runs = []
            st = rs[0]
            for a, bb in zip(rs, rs[1:] + [None]):
                if bb != a + 1:
                    runs.append((st, a))
                    st = bb
            t0, t1 = runs[0]
            nc.vector.tensor_reduce(out=s_sums[:, 2 + b:3 + b],
                                    in_=s_acc[:, t0:t1 + 1],
                                    axis=mybir.AxisListType.X, op=ALU.add)
            for (u0, u1) in runs[1:]:
                tmp = fp.tile([NLAB, 1], FP32, tag=f"tr{b}", name=f"tr{b}_{u0}")
                nc.vector.tensor_reduce(out=tmp, in_=s_acc[:, u0:u1 + 1],
                                        axis=mybir.AxisListType.X, op=ALU.add)
                nc.vector.tensor_tensor(out=s_sums[:, 2 + b:3 + b],
                                        in0=s_sums[:, 2 + b:3 + b], in1=tmp,
                                        op=ALU.add)
        # partial = tilesum + invalid count
        for b in range(B):
            nc.vector.tensor_tensor(out=s_sums[:, 2 + b:3 + b],
                                    in0=s_sums[:, 2 + b:3 + b],
                                    in1=s_sums[:, b:b + 1], op=ALU.add)
            nc.vector.memset(s_sums[:, b:b + 1], 0.0)
        cc_in = dram.tile([NLAB, 4], FP32, name="cc_in")
        cc_out = nc.dram_tensor("cc_out", [NLAB, 4], FP32, kind="Internal",
                                addr_space="Shared").ap()
        nc.sync.dma_start(out=cc_in, in_=s_sums)
        nc.gpsimd.collective_compute(
            "AllReduce", ALU.add, replica_groups=[list(range(NC))],
            ins=[cc_in[:]], outs=[cc_out], cc_dim="Partition")
        allsum = fp.tile([NLAB, 4], FP32)
        nc.sync.dma_start(out=allsum, in_=cc_out)
        nc.sync.dma_start(out=lseo, in_=allsum)
        nc.scalar.activation(s_nlse, allsum[:, 2:4], AF.Ln)
        nc.vector.tensor_scalar(out=s_nlse, in0=s_nlse, scalar1=-1.0,
                                scalar2=None, op0=ALU.mult)

        # ---- final: out - LSE, store ----
        for t in range(ntile):
            b = slot_batch[2 * t]
            seg = s_out[:, 2 * L * t:2 * L * (t + 1)]
            nc.vector.tensor_scalar(out=seg, in0=seg,
                                    scalar1=s_nlse[:, b:b + 1], scalar2=None,
                                    op0=ALU.add)
            nc.sync.dma_start(out=outd[:, 2 * L * t:2 * L * (t + 1)], in_=seg)

    return kern


def default_engine_plan(nc_getter):
    """A (static APs) on DVE; B (register-offset APs) on ACT -- ScalarE has
    the fast scalar_dynamic_offset path, VectorE dynamic APs hit a ~4us
    fallback, and GPSIMD tensor_scalar measures ~15x slower than DVE."""
    def plan(t, c, p):
        nc = nc_getter()
        engA = nc.scalar if (t + 3 * c) % 8 == 0 else nc.vector
        return engA, nc.scalar
    return plan


def kernel(**inputs) -> np.ndarray:
    hidden = np.asarray(inputs["hidden"], dtype=np.float32)
    pred_spans = np.asarray(inputs["pred_spans"]).astype(np.int64)
    span_mask = np.asarray(inputs["span_mask"]).astype(np.int32)
    W1 = np.asarray(inputs["W1"], dtype=np.float32)
    b1 = np.asarray(inputs["b1"], dtype=np.float32)
    W2 = np.asarray(inputs["W2"], dtype=np.float32)
    b2 = np.asarray(inputs["b2"], dtype=np.float32)

    spans = [(int(pred_spans[b, 0]), int(pred_spans[b, 1])) for b in range(B)]
    segs, nslot = plan_slots(spans)
    ntile = nslot // 2

    vecs = hidden[:, 1:L + 1, :]                       # [B, L, 768]
    vecsT = np.concatenate([vecs[0].T, vecs[1].T], axis=1)   # [768, 504]
    W1T = W1.T                                          # [1537, 770]
    w1iT = np.ascontiguousarray(W1T[0:HID]).astype(np.float32)
    w1jT = np.ascontiguousarray(W1T[HID:2 * HID]).astype(np.float32)
    w1c = np.ascontiguousarray(W1T[2 * HID]).astype(np.float32)
    W2T = np.ascontiguousarray(W2.T)                    # [770, 36]
    w2T6 = W2T[0:HID]
    w2Tt = np.stack([W2T[768], W2T[769], b2,
                     np.full(NLAB, BIGNEG, np.float32)], axis=0)

    maskf_full = span_mask.astype(np.float32).clip(0, 1)

    in_maps = []
    slot_maps = []
    for c in range(NC):
        sm = slot_map_for_core(segs, nslot, c)
        slot_maps.append(sm)
        myv = np.zeros((HID, nslot), np.float32)
        maskf = np.zeros((nslot, L), np.float32)
        flags = np.zeros((nslot, 2), np.float32)
        for p, ent in enumerate(sm):
            if ent is None:
                continue
            b, r = ent
            myv[:, p] = vecs[b, r]
            maskf[p] = maskf_full[r]
            flags[p, b] = 1.0
        maskb = np.zeros((2, nslot * 512), np.float32)
        for t in range(ntile):
            for sl in range(2):
                p = 2 * t + sl
                maskb[0, 1024 * t + 512 * sl:1024 * t + 512 * sl + L] = maskf[p]
                maskb[1, 1024 * t + 512 * sl:1024 * t + 512 * sl + L] = \
                    1.0 - maskf[p]
        e2f = np.zeros((128, 2), np.float32)
        for b in range(B):
            if c == 0:
                e2f[:, b] = 1.0      # row s_b lives on core 0 (slot seg start)
        in_maps.append({
            "w1iT": w1iT, "w1jT": w1jT, "w1c": w1c, "b1": b1,
            "w2T6": w2T6, "w2Tt": w2Tt,
            "vecsT": vecsT, "myvT": myv,
            "maskb": maskb, "maskf": maskf, "flags": flags, "e2f": e2f,
        })

    # ---- build program ----
    nc = bacc.Bacc("TRN2", target_bir_lowering=False, debug=False,
                   enable_asserts=False, num_devices=NC)

    def mk(name, arr, dt):
        return nc.dram_tensor(name, list(arr.shape), dt, kind="ExternalInput").ap()

    ml_bf = lambda n, a: mk(n, a, BF16)
    ml_f32 = lambda n, a: mk(n, a, FP32)
    ex = in_maps[0]
    ins_aps = {
        "w1iT": ml_bf("w1iT", ex["w1iT"]), "w1jT": ml_bf("w1jT", ex["w1jT"]),
        "w1c": ml_f32("w1c", ex["w1c"]), "b1": ml_f32("b1", ex["b1"]),
        "w2T6": ml_bf("w2T6", ex["w2T6"]), "w2Tt": ml_bf("w2Tt", ex["w2Tt"]),
        "vecsT": ml_bf("vecsT", ex["vecsT"]), "myvT": ml_bf("myvT", ex["myvT"]),
        "maskb": ml_bf("maskb", ex["maskb"]), "maskf": ml_f32("maskf", ex["maskf"]),
        "flags": ml_f32("flags", ex["flags"]), "e2f": ml_f32("e2f", ex["e2f"]),
    }
    outs_aps = {
        "out": nc.dram_tensor("out", [NLAB, nslot * L], FP32,
                              kind="ExternalOutput").ap(),
        "lse": nc.dram_tensor("lse", [NLAB, 4], FP32,
                              kind="ExternalOutput").ap(),
    }

    plan = default_engine_plan(lambda: nc)
    kern = build_kernel(spans, segs, nslot, plan)
    with tile.TileContext(nc) as t:
        kern(t, outs_aps, ins_aps)
    nc.compile()

    # bf16-cast the bf16 inputs host-side
    def cast_maps(m):
        out = {}
        for k, v in m.items():
            dt = ins_aps[k].dtype
            if dt == BF16:
                out[k] = v.astype(mybir.dt.np(BF16))
            else:
                out[k] = v.astype(np.float32)
        return out

    in_maps_c = [cast_maps(m) for m in in_maps]

    if os.environ.get("BK_BUILD_ONLY"):
        print("BUILD OK")
        return np.zeros((B, NLAB, L * L), np.float32)

    if os.environ.get("BK_SIM"):
        from concourse.bass_interp import MultiCoreSim

        sim = MultiCoreSim(nc, num_cores=NC, require_finite=False,
                           require_nnan=False)
        for c, cs in sim.cores.items():
            for name, arr in in_maps_c[c].items():
                cs.tensor(name)[:] = arr
            if nc.partition_id_tensor is not None:
                cs.tensor(nc.partition_id_tensor.name)[:] = np.array(
                    [[c]], dtype=np.uint32)
        sim.simulate(check_with_hw=False)

        class _R:
            results = [{"out": np.asarray(sim.cores[c].tensor("out")),
                        "lse": np.asarray(sim.cores[c].tensor("lse"))}
                       for c in range(NC)]
        res = _R()
    else:
        trace = bool(int(os.environ.get("BK_TRACE", "0")))
        res = run_bass_kernel_spmd(nc, in_maps_c, core_ids=list(range(NC)),
                                   trace=trace)
        if trace and res.exec_time_ns is not None:
            print(f"HW exec time: {res.exec_time_ns} ns")

    # ---- unshard ----
    out_full = np.zeros((B, NLAB, L * L), np.float32)
    for c in range(NC):
        oc = res.results[c]["out"]          # [36, nslot*252]
        for p, ent in enumerate(slot_maps[c]):
            if ent is None:
                continue
            b, r = ent
            out_full[b, :, L * r:L * (r + 1)] = oc[:, L * p:L * (p + 1)]
    return out_full
